# revision 44
# baseline (speedup 1.0000x reference)
"""Trainium2 Bass kernel for nn_GAT_GraphSAGE (N=12000, E=192000, F=35, B=64).

Sharding: the attention "row" dimension (K_new index b, the softmax row) is
sharded 1500 rows/core across 8 cores.  The tiny projections Q/K_new/V are
computed on the host and shipped as f16; the only collective is a 2-piece
AllGather of the f16 node features h.  SAGEConv aggregation runs as a dense
matmul against a host-precomputed [12032, 1536] f16 adjacency
(degree-reciprocal folded in, rows permuted to the AllGather piece layout),
streamed from HBM and contracted chunk-by-chunk on the PE.  Global max-pool +
MLP head run per-core on that core's 8 graphs.

All big matmuls are f16 (moving operand streams at 2 elem/cycle, and f16
keeps the PE HAM clock warm — fp32r measured ~2-4x slower per MM).  Scores
matmuls alternate PE row-groups 0/64 (K=35 fits in half the array), so
Q^T/K_new^T are duplicated at partitions 64:99 host-side.  exp() runs on ACT
in [128, 3*512] PSUM spans — ACT is the phase bottleneck at ~143us.
"""
import math
import numpy as np

N, E, F, B = 12000, 192000, 35, 64
NCORE = 8
ROWS = N // NCORE            # 1500
ICH = 512
NI = 3
IPAD = ICH * NI              # 1536
JT = 94                      # j chunks of 128
JPAD = JT * 128              # 12032
DBLK = 12                    # dst blocks (128 each) per core
GB = B // NCORE              # 8 graphs per core
HB = 40                      # h row padded to 40 f16 (80B)
GRAPH_BOUNDS = [int(math.ceil(g * (N / B))) for g in range(GB + 1)]
F1 = F + 1
AG0 = 1024                   # rows in first AllGather piece (i-chunks 0,1)
AG1 = ROWS - AG0             # 476 rows in second piece
NPAD = JT * 128              # padded h_full rows (12032)
PREF = 45                    # adjacency chunks prefetched during attention
QDMA = 8                     # QTd input DMA split (pipelines with attention)
# h_full row layout after the two chunked AllGathers (concat per piece):
#   src (c, r): r < AG0  -> c*AG0 + r ; else NCORE*AG0 + c*AG1 + (r - AG0)


# --------------------------------------------------------------------------
# host-side preprocessing
# --------------------------------------------------------------------------

def _host_qkv(p, f16):
    """Q, K_new (pre-scaled), V for the zero-padded node set [NPAD+, F]."""
    f64 = np.float64
    x = np.zeros((NCORE * ROWS + IPAD, F), f64)   # window pad for last core
    x[:N] = np.asarray(p['x'], f64)
    Wq, bq = p['Wq'].astype(f64), p['bq'].astype(f64)
    Wk, bk = p['Wk'].astype(f64), p['bk'].astype(f64)
    Wv, bv = p['Wv'].astype(f64), p['bv'].astype(f64)
    W3c, b3 = p['W3'][:, :, 1].astype(f64), p['b3'].astype(f64)
    W5c, b5 = p['W5'][:, :, 2].astype(f64), p['b5'].astype(f64)
    Wl, bl = p['Wl'].astype(f64), p['bl'].astype(f64)
    Wl1, Wl2, Wl3 = Wl[:, :F], Wl[:, F:2 * F], Wl[:, 2 * F:]

    Q = x @ Wq.T + bq
    K = x @ Wk.T + bk
    Kn = (np.concatenate([K @ W3c.T + b3, K @ W5c.T + b5, K], axis=1)
          @ Wl.T + bl) / np.sqrt(F)
    V = x @ Wv.T + bv

    # QTd [128, JPAD]: Q^T at partitions 0:35 and 64:99; pad cols (>=N) zero
    QTd = np.zeros((128, JPAD), np.float32)
    QT = np.zeros((F, JPAD), np.float32)
    QT[:, :N] = Q[:N].T.astype(np.float32)
    QTd[0:F] = QT
    QTd[64:64 + F] = QT

    # Vp [128, JT, F1]: V natural + ones col; rows >= N fully zero (mask)
    Vp = np.zeros((128, JT, F1), np.float32)
    Vn = V[:JPAD].astype(np.float32).reshape(JT, 128, F).transpose(1, 0, 2)
    Vp[:, :, :F] = Vn
    Vp[:, :, F] = 1.0
    pad = np.arange(JPAD).reshape(JT, 128).transpose(1, 0) >= N
    Vp[pad.nonzero()[0], pad.nonzero()[1], :] = 0.0

    # per-core: KnTd [128, IPAD] (dup), Vl [128, DBLK, F] f32
    knts, vls = [], []
    for c in range(NCORE):
        w = Kn[c * ROWS: c * ROWS + IPAD].astype(np.float32)
        kt = np.zeros((128, IPAD), np.float32)
        kt[0:F] = w.T
        kt[64:64 + F] = w.T
        knts.append(kt.astype(f16))
        vl = V[c * ROWS: c * ROWS + IPAD].astype(np.float32)
        vls.append(np.ascontiguousarray(
            vl.reshape(DBLK, 128, F).transpose(1, 0, 2)))
    return (QTd.astype(f16), knts, Vp.astype(f16), vls)


def _prep_weights(p):
    f32 = np.float32
    out = {}
    out['WllT'] = np.ascontiguousarray(p['Wll'].T).astype(f32)
    out['WlrT'] = np.ascontiguousarray(p['Wlr'].T).astype(f32)
    out['bll'] = p['bll'].astype(f32).reshape(F, 1)
    out['Wg1T'] = np.ascontiguousarray(p['Wg1'].T).astype(f32)      # [35,1500]
    bg1 = np.zeros((128, 12), f32)
    bg1.T.reshape(-1)[:1500] = p['bg1'].astype(f32)
    out['bg1'] = bg1
    w2 = np.zeros((12 * 128, 128), f32)
    w2[:1500, :] = p['Wg2'].T.astype(f32)
    out['Wg2Tr'] = np.ascontiguousarray(
        w2.reshape(12, 128, 128).transpose(1, 0, 2).reshape(128, 12 * 128))
    out['bg2'] = p['bg2'].astype(f32).reshape(128, 1)
    out['WoT'] = p['Wo'].astype(f32).reshape(1, 128).T.copy()        # [128,1]
    out['bo'] = float(np.asarray(p['bo']).reshape(-1)[0])
    return out


def _prep_adj(edge_index, f16):
    """Per-core dense [NPAD, IPAD] f16 adjacency: Adj[src_pos, dst_local] =
    multiplicity / deg(dst), rows permuted to the AllGather piece layout."""
    src = np.asarray(edge_index[0], np.int64)
    dst = np.asarray(edge_index[1], np.int64)
    deg = np.bincount(dst, minlength=N).astype(np.float64)
    recip = (1.0 / np.maximum(deg, 1.0)).astype(np.float32)

    sc = src // ROWS
    sr = src - sc * ROWS
    src_pos = np.where(sr < AG0, sc * AG0 + sr,
                       NCORE * AG0 + sc * AG1 + (sr - AG0))

    core_of = dst // ROWS
    adjs = []
    for c in range(NCORE):
        m = core_of == c
        A = np.zeros((NPAD, IPAD), np.float32)
        np.add.at(A, (src_pos[m], dst[m] - c * ROWS), recip[dst[m]])
        adjs.append(np.ascontiguousarray(A.astype(f16)))
    return adjs


# --------------------------------------------------------------------------
# device program
# --------------------------------------------------------------------------

def _emit_body(nc, tc, d):
    import concourse.tile as tile  # noqa: F401
    from concourse import mybir

    f32 = mybir.dt.float32
    f16 = mybir.dt.float16
    exp_f = mybir.ActivationFunctionType.Exp
    relu_f = mybir.ActivationFunctionType.Relu

    with tc.tile_pool(name="const", bufs=1) as constp, \
         tc.tile_pool(name="main", bufs=1) as main, \
         tc.tile_pool(name="adjr", bufs=PREF) as adjp:
        # preload the exp table set so the ~2.7us ACT_TABLE_LOAD overlaps
        # the input DMAs instead of stalling the first scores group
        warm_t = constp.tile([1, 8], f32, name="warm_t")
        nc.vector.memset(warm_t[:], 0.0)
        nc.scalar.activation(out=warm_t[:], in_=warm_t[:],
                             func=exp_f)

        # ---- attention operands (host-computed, f16) ----
        # DMA order = first-use order: group 0 needs KnT + QT piece 0, then
        # Vp for the first U accumulation.
        KnT = main.tile([128, IPAD], f16, name="KnT")
        nc.sync.dma_start(out=KnT[:], in_=d['KnTd'][:, :])
        QT = main.tile([128, JPAD], f16, name="QT")
        QW = JPAD // QDMA
        nc.sync.dma_start(out=QT[:, 0:512], in_=d['QTd'][:, 0:512])
        nc.sync.dma_start(out=QT[:, 512:QW], in_=d['QTd'][:, 512:QW])
        Vp = main.tile([128, JT, F1], f16, name="Vp")
        nc.sync.dma_start(out=Vp[:], in_=d['Vp'][:, :, :])
        nc.sync.dma_start(out=QT[:, QW:2 * QW], in_=d['QTd'][:, QW:2 * QW])
        Vl = main.tile([128, DBLK, F], f32, name="Vl")
        nc.sync.dma_start(out=Vl[:], in_=d['Vl'][:, :, :])
        ident_t = constp.tile([128, 128], f32, name="ident_t")
        nc.sync.dma_start(out=ident_t[:], in_=d['ident'][:, :])
        for q in range(2, QDMA):
            nc.sync.dma_start(out=QT[:, q * QW:(q + 1) * QW],
                              in_=d['QTd'][:, q * QW:(q + 1) * QW])
        zeros_t = constp.tile([32, HB], f16, name="zeros_t")
        nc.vector.memset(zeros_t[:], 0.0)
        nc.sync.dma_start(out=d['h_full'][N:NPAD, :], in_=zeros_t[:])

        hnat = main.tile([128, DBLK, F], f32, name="hnat")
        hnatb = main.tile([128, DBLK, HB], f16, name="hnatb")
        nc.vector.memset(hnatb[:], 0.0)
        hfullA = main.tile([128, 64, HB], f16, name="hfullA")
        hfullB = main.tile([128, JT - 64, HB], f16, name="hfullB")

        # ---- adjacency prefetch ring ----
        # Capped at PREF during attention: the SP DMA queue is FIFO, so a
        # ring-slot-blocked Adj DMA ahead of an h_loc write would deadlock
        # the AllGather.
        adj_tiles = {}
        pref_state = [0]

        def emit_adj(limit, eng=None):
            j = pref_state[0]
            if j >= limit:
                return
            t = adjp.tile([128, IPAD], f16, tag="adj", name="adjt")
            (eng or nc.sync).dma_start(
                out=t[:], in_=d['Adj'][j * 128:(j + 1) * 128, :])
            adj_tiles[j] = t
            pref_state[0] = j + 1

        # ---------------- attention ----------------
        # j-chunks in groups of 3: one ACT exp covers [128, 1536] (3 PSUM
        # banks) — ACT is the bottleneck engine of this phase.  Scores MMs
        # alternate row-groups 0/64 by j parity.
        GROUPS = [(g * 3, 3) for g in range(JT // 3)]
        if JT % 3:
            GROUPS.append((JT - JT % 3, JT % 3))
        hT = main.tile([F, IPAD], f16, name="hT")
        with tc.tile_pool(name="mm1p", bufs=2, space="PSUM") as mm1p, \
             tc.tile_pool(name="Up", bufs=1, space="PSUM") as Upp, \
             tc.tile_pool(name="tp", bufs=1, space="PSUM") as tpp, \
             tc.tile_pool(name="esb", bufs=3) as esb, \
             tc.tile_pool(name="usb", bufs=2) as usb, \
             tc.tile_pool(name="hsm", bufs=4) as hsmall:

            def emit_tail(ci, Usb):
                # normalize + residual + relu -> h natural tiles (f32 + f16),
                # plus the hT transposes for the SAGE lin_r term.  Deferred
                # past the next ci's first scores groups so the PE transposes
                # don't stall the exp pipeline at ci boundaries.
                for t in range(4):
                    blk = ci * 4 + t
                    up = tpp.tile([128, F1], f32, space="PSUM", tag="unat",
                                  name="up")
                    nc.tensor.transpose(out=up[:],
                                        in_=Usb[:, t * 128:(t + 1) * 128],
                                        identity=ident_t[:F1, :F1])
                    rec = hsmall.tile([128, 1], f32, tag="rec", name="rec")
                    nc.vector.reciprocal(out=rec[:], in_=up[:, F:F1])
                    hh = hsmall.tile([128, F], f32, tag="hh", name="hh")
                    nc.vector.scalar_tensor_tensor(
                        out=hh[:], in0=up[:, :F], scalar=rec[:],
                        in1=Vl[:, blk, :], op0=mybir.AluOpType.mult,
                        op1=mybir.AluOpType.add)
                    nc.vector.tensor_scalar_max(out=hnat[:, blk, :],
                                                in0=hh[:], scalar1=0.0)
                    nc.vector.tensor_copy(out=hnatb[:, blk, :F],
                                          in_=hnat[:, blk, :])
                    lo = blk * 128
                    nrows = min(128, max(0, ROWS - lo))
                    if nrows > 0:
                        nc.sync.dma_start(
                            out=d['h_loc'][lo:lo + nrows, :],
                            in_=hnatb[:nrows, blk, :])
                    ht_ps = tpp.tile([F, 128], f32, space="PSUM", tag="unat",
                                     name="htps")
                    nc.tensor.transpose(out=ht_ps[:], in_=hnat[:, blk, :],
                                        identity=ident_t[:])
                    nc.vector.tensor_copy(
                        out=hT[:, blk * 128:(blk + 1) * 128], in_=ht_ps[:])
                # first AllGather piece (rows 0:1024) issues while the last
                # i-chunk is still computing — hides most of the collective.
                if ci == 1:
                    nc.gpsimd.collective_compute(
                        "AllGather", mybir.AluOpType.bypass,
                        replica_groups=[list(range(NCORE))],
                        ins=[d['h_loc'][0:AG0, :]],
                        outs=[d['h_full'][0:NCORE * AG0, :]])
                    nc.sync.dma_start(
                        out=hfullA[:, :, :],
                        in_=d['h_full'][0:NCORE * AG0, :].rearrange(
                            "(j p) e -> p j e", p=128))

            pending_tail = [None]
            for ci in range(NI):
                Ups = Upp.tile([F1, ICH], f32, space="PSUM", tag="U",
                               name="Ups")
                prev = None  # (exp_tile, j0, glen)
                for gi, (j0, glen) in enumerate(GROUPS):
                    ps = mm1p.tile([128, 3 * ICH], f32, space="PSUM", tag="s",
                                   name="pss")
                    for k in range(glen):
                        j = j0 + k
                        rp = 64 if (j & 1) else 0
                        nc.tensor.matmul(
                            out=ps[:, k * ICH:(k + 1) * ICH],
                            lhsT=QT[rp:rp + F, j * 128:(j + 1) * 128],
                            rhs=KnT[rp:rp + F, ci * ICH:(ci + 1) * ICH],
                            start=True, stop=True, tile_position=(rp, 0))
                    et = esb.tile([128, 3 * ICH], f16, tag="e", name="et")
                    nc.scalar.activation(out=et[:, :glen * ICH],
                                         in_=ps[:, :glen * ICH], func=exp_f)
                    if prev is not None:
                        pe, pj0, pglen = prev
                        for k in range(pglen):
                            nc.tensor.matmul(
                                out=Ups[:], lhsT=Vp[:, pj0 + k, :],
                                rhs=pe[:, k * ICH:(k + 1) * ICH],
                                start=(pj0 + k == 0), stop=False,
                                skip_group_check=True)
                    prev = (et, j0, glen)
                    emit_adj(PREF)
                    if gi == 2 and pending_tail[0] is not None:
                        pending_tail[0]()
                        pending_tail[0] = None
                pe, pj0, pglen = prev
                for k in range(pglen):
                    nc.tensor.matmul(out=Ups[:], lhsT=Vp[:, pj0 + k, :],
                                     rhs=pe[:, k * ICH:(k + 1) * ICH],
                                     start=False, stop=(k == pglen - 1),
                                     skip_group_check=True)
                Usb = usb.tile([F1, ICH], f32, tag="usb", name="Usb")
                nc.vector.tensor_copy(out=Usb[:], in_=Ups[:])
                pending_tail[0] = (lambda ci=ci, Usb=Usb: emit_tail(ci, Usb))
            pending_tail[0]()

        # ---------------- AllGather h: second piece (rows 1024:1500) -----
        base = NCORE * AG0
        nc.gpsimd.collective_compute(
            "AllGather", mybir.AluOpType.bypass,
            replica_groups=[list(range(NCORE))],
            ins=[d['h_loc'][AG0:ROWS, :]],
            outs=[d['h_full'][base:N, :]])

        # ---------------- SAGE aggregation: dense Adj matmul ----------------
        # chunks 0..63 depend only on the first AllGather piece, so they run
        # concurrently with the second AllGather.
        aggdT = main.tile([F, IPAD], f16, name="aggdT")
        h2T = main.tile([F, IPAD], f16, name="h2T")
        with tc.tile_pool(name="agp", bufs=1, space="PSUM") as agp:
            aggps = [agp.tile([F, ICH], f32, space="PSUM", tag=f"ag{ci}",
                              name=f"aggps{ci}") for ci in range(NI)]
            for j in range(JT):
                if j == 19:
                    nc.sync.dma_start(
                        out=hfullB[:, :, :],
                        in_=d['h_full'][base:NPAD, :].rearrange(
                            "(j p) e -> p j e", p=128))
                at = adj_tiles.pop(j)
                hsrc = hfullA[:, j, :F] if j < 64 else hfullB[:, j - 64, :F]
                for ci in range(NI):
                    nc.tensor.matmul(out=aggps[ci][:],
                                     lhsT=hsrc,
                                     rhs=at[:, ci * ICH:(ci + 1) * ICH],
                                     start=(j == 0), stop=(j == JT - 1),
                                     skip_group_check=True)
                # alternate the refills across both HWDGE rings (SP + ACT):
                # ACT is idle here, and two rings roughly double stream BW
                emit_adj(JT, eng=nc.scalar if (j & 1) else nc.sync)
            for ci in range(NI):
                nc.vector.tensor_copy(out=aggdT[:, ci * ICH:(ci + 1) * ICH],
                                      in_=aggps[ci][:])

        # ---------------- SAGE linear + pool + MLP ----------------
        with tc.tile_pool(name="mlpw", bufs=1) as mlpw, \
             tc.tile_pool(name="mlps", bufs=2) as mlps, \
             tc.tile_pool(name="mlpp", bufs=2, space="PSUM") as mlpp:
            WllT_t = mlpw.tile([F, F], f16, name="WllT_t")
            nc.sync.dma_start(out=WllT_t[:], in_=d['WllT'][:, :])
            WlrT_t = mlpw.tile([F, F], f16, name="WlrT_t")
            nc.sync.dma_start(out=WlrT_t[:], in_=d['WlrT'][:, :])
            bll_t = mlpw.tile([F, 1], f32, name="bll_t")
            nc.sync.dma_start(out=bll_t[:], in_=d['bll'][:, :])
            Wg1T_t = mlpw.tile([F, 1500], f32, name="Wg1T_t")
            nc.sync.dma_start(out=Wg1T_t[:], in_=d['Wg1T'][:, :])
            bg1_t = mlpw.tile([128, 12], f32, name="bg1_t")
            nc.sync.dma_start(out=bg1_t[:], in_=d['bg1'][:, :])
            Wg2_t = mlpw.tile([128, 12 * 128], f32, name="Wg2_t")
            nc.sync.dma_start(out=Wg2_t[:], in_=d['Wg2Tr'][:, :])
            bg2_t = mlpw.tile([128, 1], f32, name="bg2_t")
            nc.sync.dma_start(out=bg2_t[:], in_=d['bg2'][:, :])
            WoT_t = mlpw.tile([128, 1], f32, name="WoT_t")
            nc.sync.dma_start(out=WoT_t[:], in_=d['WoT'][:, :])

            for ci in range(NI):
                ps = mlpp.tile([F, ICH], f32, space="PSUM", tag="h2",
                               name="psh2")
                nc.tensor.matmul(out=ps[:], lhsT=WllT_t[:],
                                 rhs=aggdT[:, ci * ICH:(ci + 1) * ICH],
                                 start=True, stop=False, skip_group_check=True)
                nc.tensor.matmul(out=ps[:], lhsT=WlrT_t[:],
                                 rhs=hT[:, ci * ICH:(ci + 1) * ICH],
                                 start=False, stop=True, skip_group_check=True)
                nc.scalar.activation(out=h2T[:, ci * ICH:(ci + 1) * ICH],
                                     in_=ps[:], func=relu_f, bias=bll_t[:])

            gT = mlps.tile([F, GB], f32, name="gT")
            for g in range(GB):
                lo, hi = GRAPH_BOUNDS[g], GRAPH_BOUNDS[g + 1]
                nc.vector.tensor_reduce(out=gT[:, g:g + 1], in_=h2T[:, lo:hi],
                                        axis=mybir.AxisListType.X,
                                        op=mybir.AluOpType.max)
            g1T = mlps.tile([128, 12, GB], f32, name="g1T")
            for j in range(12):
                w = min(128, 1500 - j * 128)
                ps = mlpp.tile([128, GB], f32, space="PSUM", tag="g1",
                               name="psg1")
                nc.tensor.matmul(out=ps[:w, :],
                                 lhsT=Wg1T_t[:, j * 128:j * 128 + w],
                                 rhs=gT[:], start=True, stop=True)
                if w < 128:
                    nc.vector.memset(g1T[:, j, :], 0.0)
                nc.scalar.activation(out=g1T[:w, j, :], in_=ps[:w, :],
                                     func=relu_f, bias=bg1_t[:w, j:j + 1])
            g2ps = mlpp.tile([128, GB], f32, space="PSUM", tag="g2",
                             name="g2ps")
            for j in range(12):
                nc.tensor.matmul(out=g2ps[:],
                                 lhsT=Wg2_t[:, j * 128:(j + 1) * 128],
                                 rhs=g1T[:, j, :], start=(j == 0),
                                 stop=(j == 11), skip_group_check=True)
            g2sb = mlps.tile([128, GB], f32, name="g2sb")
            nc.vector.tensor_scalar_add(out=g2sb[:], in0=g2ps[:],
                                        scalar1=bg2_t[:])
            ops = mlpp.tile([1, GB], f32, space="PSUM", tag="o", name="ops")
            nc.tensor.matmul(out=ops[:], lhsT=WoT_t[:], rhs=g2sb[:],
                             start=True, stop=True)
            osb = mlps.tile([1, GB], f32, name="osb")
            nc.vector.tensor_scalar_add(out=osb[:], in0=ops[:],
                                        scalar1=float(d['bo_const']))
            nc.sync.dma_start(out=d['out8'][:, :], in_=osb[:])


def _build_program(bo_const):
    import concourse.tile as tile
    from concourse import bacc, mybir

    f32 = mybir.dt.float32
    f16 = mybir.dt.float16
    nc = bacc.Bacc("TRN2", target_bir_lowering=False, debug=False,
                   num_devices=NCORE)

    d = {}

    def dram_in(name, shape, dt=f32):
        d[name] = nc.dram_tensor(name, list(shape), dt, kind="ExternalInput")

    dram_in("QTd", (128, JPAD), f16)
    dram_in("KnTd", (128, IPAD), f16)
    dram_in("Vp", (128, JT, F1), f16)
    dram_in("Vl", (128, DBLK, F))
    dram_in("WllT", (F, F), f16)
    dram_in("WlrT", (F, F), f16)
    dram_in("bll", (F, 1))
    dram_in("Wg1T", (F, 1500))
    dram_in("bg1", (128, 12))
    dram_in("Wg2Tr", (128, 12 * 128))
    dram_in("bg2", (128, 1))
    dram_in("WoT", (128, 1))
    dram_in("ident", (128, 128))
    dram_in("Adj", (NPAD, IPAD), f16)
    d['out8'] = nc.dram_tensor("out8", [1, GB], f32, kind="ExternalOutput")
    d['h_loc'] = nc.dram_tensor("h_loc", [ROWS, HB], f16)
    d['h_full'] = nc.dram_tensor("h_full", [NPAD, HB], f16,
                                 addr_space="Shared")
    d['bo_const'] = bo_const

    with tile.TileContext(nc) as tc:
        _emit_body(nc, tc, d)

    nc.compile()
    return nc


# --------------------------------------------------------------------------
# entry point
# --------------------------------------------------------------------------

_CACHE = {}


def _make_in_maps(inputs):
    from concourse import mybir

    f16 = np.float16
    edge_index = np.asarray(inputs['edge_index'])
    w = _prep_weights(inputs)
    QTd, knts, Vp, vls = _host_qkv(inputs, f16)
    adjs = _prep_adj(edge_index, f16)
    ident = np.eye(128, dtype=np.float32)
    common = dict(
        QTd=QTd, Vp=Vp,
        WllT=w['WllT'].astype(f16), WlrT=w['WlrT'].astype(f16),
        bll=w['bll'], Wg1T=w['Wg1T'], bg1=w['bg1'], Wg2Tr=w['Wg2Tr'],
        bg2=w['bg2'], WoT=w['WoT'], ident=ident)
    in_maps = []
    for c in range(NCORE):
        m = dict(common)
        m['KnTd'] = knts[c]
        m['Vl'] = vls[c]
        m['Adj'] = adjs[c]
        in_maps.append(m)
    return in_maps, w['bo']


def kernel(**inputs):
    from concourse.bass_utils import run_bass_kernel_spmd

    in_maps, bo = _make_in_maps(inputs)
    key = ('prog', bo)
    if key not in _CACHE:
        _CACHE[key] = _build_program(bo)
    nc = _CACHE[key]

    res = run_bass_kernel_spmd(nc, in_maps, list(range(NCORE)))
    global LAST_RESULT
    LAST_RESULT = res
    out = np.zeros((B, 1), np.float32)
    for c in range(NCORE):
        out[c * GB:(c + 1) * GB, 0] = res.results[c]['out8'].reshape(-1)
    return out


LAST_RESULT = None


# revision 45
# speedup vs baseline: 1.0832x; 1.0832x over previous
"""Trainium2 Bass kernel for nn_GAT_GraphSAGE (N=12000, E=192000, F=35, B=64).

Sharding: the attention "row" dimension (K_new index b, the softmax row) is
sharded 1500 rows/core across 8 cores.  The tiny projections Q/K_new/V are
computed on the host and shipped as f16; the only collective is a 2-piece
AllGather of the f16 node features h.  SAGEConv aggregation runs as a dense
matmul against a host-precomputed [12032, 1536] f16 adjacency
(degree-reciprocal folded in, rows permuted to the AllGather piece layout),
streamed from HBM and contracted chunk-by-chunk on the PE.  Global max-pool +
MLP head run per-core on that core's 8 graphs.

All big matmuls are f16 (moving operand streams at 2 elem/cycle, and f16
keeps the PE HAM clock warm — fp32r measured ~2-4x slower per MM).  Scores
matmuls alternate PE row-groups 0/64 (K=35 fits in half the array), so
Q^T/K_new^T are duplicated at partitions 64:99 host-side.  exp() runs on ACT
in [128, 3*512] PSUM spans — ACT is the phase bottleneck at ~143us.
"""
import math
import numpy as np

N, E, F, B = 12000, 192000, 35, 64
NCORE = 8
ROWS = N // NCORE            # 1500
ICH = 512
NI = 3
IPAD = ICH * NI              # 1536
JT = 94                      # j chunks of 128
JPAD = JT * 128              # 12032
DBLK = 12                    # dst blocks (128 each) per core
GB = B // NCORE              # 8 graphs per core
HB = 40                      # h row padded to 40 f16 (80B)
GRAPH_BOUNDS = [int(math.ceil(g * (N / B))) for g in range(GB + 1)]
F1 = F + 1
AG0 = 1024                   # rows in first AllGather piece (i-chunks 0,1)
AG1 = ROWS - AG0             # 476 rows in second piece
NPAD = JT * 128              # padded h_full rows (12032)
PREF = 45                    # adjacency chunks prefetched during attention
QDMA = 8                     # QTd input DMA split (pipelines with attention)
# h_full row layout after the two chunked AllGathers (concat per piece):
#   src (c, r): r < AG0  -> c*AG0 + r ; else NCORE*AG0 + c*AG1 + (r - AG0)


# --------------------------------------------------------------------------
# host-side preprocessing
# --------------------------------------------------------------------------

def _host_qkv(p, f16):
    """Q, K_new (pre-scaled), V for the zero-padded node set [NPAD+, F]."""
    f64 = np.float64
    x = np.zeros((NCORE * ROWS + IPAD, F), f64)   # window pad for last core
    x[:N] = np.asarray(p['x'], f64)
    Wq, bq = p['Wq'].astype(f64), p['bq'].astype(f64)
    Wk, bk = p['Wk'].astype(f64), p['bk'].astype(f64)
    Wv, bv = p['Wv'].astype(f64), p['bv'].astype(f64)
    W3c, b3 = p['W3'][:, :, 1].astype(f64), p['b3'].astype(f64)
    W5c, b5 = p['W5'][:, :, 2].astype(f64), p['b5'].astype(f64)
    Wl, bl = p['Wl'].astype(f64), p['bl'].astype(f64)
    Wl1, Wl2, Wl3 = Wl[:, :F], Wl[:, F:2 * F], Wl[:, 2 * F:]

    Q = x @ Wq.T + bq
    K = x @ Wk.T + bk
    Kn = (np.concatenate([K @ W3c.T + b3, K @ W5c.T + b5, K], axis=1)
          @ Wl.T + bl) / np.sqrt(F)
    V = x @ Wv.T + bv

    # QTd [128, JPAD]: Q^T at partitions 0:35 and 64:99; pad cols (>=N) zero
    QTd = np.zeros((128, JPAD), np.float32)
    QT = np.zeros((F, JPAD), np.float32)
    QT[:, :N] = Q[:N].T.astype(np.float32)
    QTd[0:F] = QT
    QTd[64:64 + F] = QT

    # Vp [128, JT, F1]: V natural + ones col; rows >= N fully zero (mask)
    Vp = np.zeros((128, JT, F1), np.float32)
    Vn = V[:JPAD].astype(np.float32).reshape(JT, 128, F).transpose(1, 0, 2)
    Vp[:, :, :F] = Vn
    Vp[:, :, F] = 1.0
    pad = np.arange(JPAD).reshape(JT, 128).transpose(1, 0) >= N
    Vp[pad.nonzero()[0], pad.nonzero()[1], :] = 0.0

    # per-core: KnTd [128, IPAD] (dup), Vl [128, DBLK, F] f32
    knts, vls = [], []
    for c in range(NCORE):
        w = Kn[c * ROWS: c * ROWS + IPAD].astype(np.float32)
        kt = np.zeros((128, IPAD), np.float32)
        kt[0:F] = w.T
        kt[64:64 + F] = w.T
        knts.append(kt.astype(f16))
        vl = V[c * ROWS: c * ROWS + IPAD].astype(np.float32)
        vls.append(np.ascontiguousarray(
            vl.reshape(DBLK, 128, F).transpose(1, 0, 2)))
    return (QTd.astype(f16), knts, Vp.astype(f16), vls)


def _prep_weights(p):
    f32 = np.float32
    out = {}
    out['WllT'] = np.ascontiguousarray(p['Wll'].T).astype(f32)
    out['WlrT'] = np.ascontiguousarray(p['Wlr'].T).astype(f32)
    out['bll'] = p['bll'].astype(f32).reshape(F, 1)
    out['Wg1T'] = np.ascontiguousarray(p['Wg1'].T).astype(f32)      # [35,1500]
    bg1 = np.zeros((128, 12), f32)
    bg1.T.reshape(-1)[:1500] = p['bg1'].astype(f32)
    out['bg1'] = bg1
    w2 = np.zeros((12 * 128, 128), f32)
    w2[:1500, :] = p['Wg2'].T.astype(f32)
    out['Wg2Tr'] = np.ascontiguousarray(
        w2.reshape(12, 128, 128).transpose(1, 0, 2).reshape(128, 12 * 128))
    out['bg2'] = p['bg2'].astype(f32).reshape(128, 1)
    out['WoT'] = p['Wo'].astype(f32).reshape(1, 128).T.copy()        # [128,1]
    out['bo'] = float(np.asarray(p['bo']).reshape(-1)[0])
    return out


def _prep_adj(edge_index, f16):
    """Per-core dense [NPAD, IPAD] f16 adjacency: Adj[src_pos, dst_local] =
    multiplicity / deg(dst), rows permuted to the AllGather piece layout."""
    src = np.asarray(edge_index[0], np.int64)
    dst = np.asarray(edge_index[1], np.int64)
    deg = np.bincount(dst, minlength=N).astype(np.float64)
    recip = (1.0 / np.maximum(deg, 1.0)).astype(np.float32)

    sc = src // ROWS
    sr = src - sc * ROWS
    src_pos = np.where(sr < AG0, sc * AG0 + sr,
                       NCORE * AG0 + sc * AG1 + (sr - AG0))

    core_of = dst // ROWS
    adjs = []
    for c in range(NCORE):
        m = core_of == c
        A = np.zeros((NPAD, IPAD), np.float32)
        np.add.at(A, (src_pos[m], dst[m] - c * ROWS), recip[dst[m]])
        adjs.append(np.ascontiguousarray(A.astype(f16)))
    return adjs


# --------------------------------------------------------------------------
# device program
# --------------------------------------------------------------------------

def _emit_body(nc, tc, d):
    import concourse.tile as tile  # noqa: F401
    from concourse import mybir

    f32 = mybir.dt.float32
    f16 = mybir.dt.float16
    exp_f = mybir.ActivationFunctionType.Exp
    relu_f = mybir.ActivationFunctionType.Relu

    with tc.tile_pool(name="const", bufs=1) as constp, \
         tc.tile_pool(name="main", bufs=1) as main, \
         tc.tile_pool(name="adjr", bufs=PREF) as adjp:
        # preload the exp table set so the ~2.7us ACT_TABLE_LOAD overlaps
        # the input DMAs instead of stalling the first scores group
        warm_t = constp.tile([1, 8], f32, name="warm_t")
        nc.vector.memset(warm_t[:], 0.0)
        nc.scalar.activation(out=warm_t[:], in_=warm_t[:],
                             func=exp_f)

        # ---- attention operands (host-computed, f16) ----
        # DMA order = first-use order: group 0 needs KnT + QT piece 0, then
        # Vp for the first U accumulation.
        KnT = main.tile([128, IPAD], f16, name="KnT")
        nc.sync.dma_start(out=KnT[:], in_=d['KnTd'][:, :])
        QT = main.tile([128, JPAD], f16, name="QT")
        QW = JPAD // QDMA
        nc.sync.dma_start(out=QT[:, 0:512], in_=d['QTd'][:, 0:512])
        nc.sync.dma_start(out=QT[:, 512:QW], in_=d['QTd'][:, 512:QW])
        Vp = main.tile([128, JT, F1], f16, name="Vp")
        nc.sync.dma_start(out=Vp[:], in_=d['Vp'][:, :, :])
        nc.sync.dma_start(out=QT[:, QW:2 * QW], in_=d['QTd'][:, QW:2 * QW])
        Vl = main.tile([128, DBLK, F], f32, name="Vl")
        nc.sync.dma_start(out=Vl[:], in_=d['Vl'][:, :, :])
        ident_t = constp.tile([128, 128], f32, name="ident_t")
        nc.sync.dma_start(out=ident_t[:], in_=d['ident'][:, :])
        for q in range(2, QDMA):
            nc.sync.dma_start(out=QT[:, q * QW:(q + 1) * QW],
                              in_=d['QTd'][:, q * QW:(q + 1) * QW])
        zeros_t = constp.tile([32, HB], f16, name="zeros_t")
        nc.vector.memset(zeros_t[:], 0.0)
        nc.sync.dma_start(out=d['h_full'][N:NPAD, :], in_=zeros_t[:])

        hnat = main.tile([128, DBLK, F], f32, name="hnat")
        hnatb = main.tile([128, DBLK, HB], f16, name="hnatb")
        nc.vector.memset(hnatb[:], 0.0)
        hfullA = main.tile([128, 64, HB], f16, name="hfullA")
        hfullB = main.tile([128, JT - 64, HB], f16, name="hfullB")

        # ---- adjacency prefetch ring ----
        # Capped at PREF during attention: the SP DMA queue is FIFO, so a
        # ring-slot-blocked Adj DMA ahead of an h_loc write would deadlock
        # the AllGather.
        adj_tiles = {}
        pref_state = [0]

        def emit_adj(limit, eng=None):
            j = pref_state[0]
            if j >= limit:
                return
            t = adjp.tile([128, IPAD], f16, tag="adj", name="adjt")
            (eng or nc.sync).dma_start(
                out=t[:], in_=d['Adj'][j * 128:(j + 1) * 128, :])
            adj_tiles[j] = t
            pref_state[0] = j + 1

        # ---------------- attention ----------------
        # j-chunks in groups of 3: one ACT exp covers [128, 1536] (3 PSUM
        # banks) — ACT is the bottleneck engine of this phase.  Scores MMs
        # alternate row-groups 0/64 by j parity.
        GROUPS = [(g * 3, 3) for g in range(JT // 3)]
        if JT % 3:
            GROUPS.append((JT - JT % 3, JT % 3))
        hT = main.tile([F, IPAD], f16, name="hT")
        with tc.tile_pool(name="mm1p", bufs=2, space="PSUM") as mm1p, \
             tc.tile_pool(name="Up", bufs=1, space="PSUM") as Upp, \
             tc.tile_pool(name="tp", bufs=1, space="PSUM") as tpp, \
             tc.tile_pool(name="esb", bufs=3) as esb, \
             tc.tile_pool(name="usb", bufs=2) as usb, \
             tc.tile_pool(name="hsm", bufs=4) as hsmall:

            def emit_tail(ci, Usb):
                # normalize + residual + relu -> h natural tiles (f32 + f16),
                # plus the hT transposes for the SAGE lin_r term.  Deferred
                # past the next ci's first scores groups so the PE transposes
                # don't stall the exp pipeline at ci boundaries.
                for t in range(4):
                    blk = ci * 4 + t
                    up = tpp.tile([128, F1], f32, space="PSUM", tag="unat",
                                  name="up")
                    nc.tensor.transpose(out=up[:],
                                        in_=Usb[:, t * 128:(t + 1) * 128],
                                        identity=ident_t[:F1, :F1])
                    rec = hsmall.tile([128, 1], f32, tag="rec", name="rec")
                    nc.vector.reciprocal(out=rec[:], in_=up[:, F:F1])
                    hh = hsmall.tile([128, F], f32, tag="hh", name="hh")
                    nc.vector.scalar_tensor_tensor(
                        out=hh[:], in0=up[:, :F], scalar=rec[:],
                        in1=Vl[:, blk, :], op0=mybir.AluOpType.mult,
                        op1=mybir.AluOpType.add)
                    nc.vector.tensor_scalar_max(out=hnat[:, blk, :],
                                                in0=hh[:], scalar1=0.0)
                    nc.vector.tensor_copy(out=hnatb[:, blk, :F],
                                          in_=hnat[:, blk, :])
                    lo = blk * 128
                    nrows = min(128, max(0, ROWS - lo))
                    if nrows > 0:
                        nc.sync.dma_start(
                            out=d['h_loc'][lo:lo + nrows, :],
                            in_=hnatb[:nrows, blk, :])
                    ht_ps = tpp.tile([F, 128], f32, space="PSUM", tag="unat",
                                     name="htps")
                    nc.tensor.transpose(out=ht_ps[:], in_=hnat[:, blk, :],
                                        identity=ident_t[:])
                    nc.vector.tensor_copy(
                        out=hT[:, blk * 128:(blk + 1) * 128], in_=ht_ps[:])
                # first AllGather piece (rows 0:1024) issues while the last
                # i-chunk is still computing — hides most of the collective.
                if ci == 1:
                    nc.gpsimd.collective_compute(
                        "AllGather", mybir.AluOpType.bypass,
                        replica_groups=[list(range(NCORE))],
                        ins=[d['h_loc'][0:AG0, :]],
                        outs=[d['h_full'][0:NCORE * AG0, :]])
                    nc.sync.dma_start(
                        out=hfullA[:, :, :],
                        in_=d['h_full'][0:NCORE * AG0, :].rearrange(
                            "(j p) e -> p j e", p=128))

            pending_tail = [None]
            for ci in range(NI):
                Ups = Upp.tile([F1, ICH], f32, space="PSUM", tag="U",
                               name="Ups")
                prev = None  # (exp_tile, j0, glen)
                for gi, (j0, glen) in enumerate(GROUPS):
                    ps = mm1p.tile([128, 3 * ICH], f32, space="PSUM", tag="s",
                                   name="pss")
                    for k in range(glen):
                        j = j0 + k
                        rp = 64 if (j & 1) else 0
                        nc.tensor.matmul(
                            out=ps[:, k * ICH:(k + 1) * ICH],
                            lhsT=QT[rp:rp + F, j * 128:(j + 1) * 128],
                            rhs=KnT[rp:rp + F, ci * ICH:(ci + 1) * ICH],
                            start=True, stop=True, tile_position=(rp, 0))
                    et = esb.tile([128, 3 * ICH], f16, tag="e", name="et")
                    nc.scalar.activation(out=et[:, :glen * ICH],
                                         in_=ps[:, :glen * ICH], func=exp_f)
                    if prev is not None:
                        pe, pj0, pglen = prev
                        for k in range(pglen):
                            nc.tensor.matmul(
                                out=Ups[:], lhsT=Vp[:, pj0 + k, :],
                                rhs=pe[:, k * ICH:(k + 1) * ICH],
                                start=(pj0 + k == 0), stop=False,
                                skip_group_check=True)
                    prev = (et, j0, glen)
                    emit_adj(PREF)
                    if gi == 2 and pending_tail[0] is not None:
                        pending_tail[0]()
                        pending_tail[0] = None
                pe, pj0, pglen = prev
                for k in range(pglen):
                    nc.tensor.matmul(out=Ups[:], lhsT=Vp[:, pj0 + k, :],
                                     rhs=pe[:, k * ICH:(k + 1) * ICH],
                                     start=False, stop=(k == pglen - 1),
                                     skip_group_check=True)
                Usb = usb.tile([F1, ICH], f32, tag="usb", name="Usb")
                nc.vector.tensor_copy(out=Usb[:], in_=Ups[:])
                pending_tail[0] = (lambda ci=ci, Usb=Usb: emit_tail(ci, Usb))
            pending_tail[0]()

        # ---------------- AllGather h: second piece (rows 1024:1500) -----
        base = NCORE * AG0
        nc.gpsimd.collective_compute(
            "AllGather", mybir.AluOpType.bypass,
            replica_groups=[list(range(NCORE))],
            ins=[d['h_loc'][AG0:ROWS, :]],
            outs=[d['h_full'][base:N, :]])
        nc.sync.dma_start(out=hfullB[:, :, :],
                          in_=d['h_full'][base:NPAD, :].rearrange(
                              "(j p) e -> p j e", p=128))

        # ---------------- SAGE aggregation: dense Adj matmul ----------------
        # chunks 0..63 depend only on the first AllGather piece, so they run
        # concurrently with the second AllGather.
        aggdT = main.tile([F, IPAD], f16, name="aggdT")
        h2T = main.tile([F, IPAD], f16, name="h2T")
        with tc.tile_pool(name="agp", bufs=1, space="PSUM") as agp:
            aggps = [agp.tile([F, ICH], f32, space="PSUM", tag=f"ag{ci}",
                              name=f"aggps{ci}") for ci in range(NI)]
            for j in range(JT):
                at = adj_tiles.pop(j)
                hsrc = hfullA[:, j, :F] if j < 64 else hfullB[:, j - 64, :F]
                for ci in range(NI):
                    nc.tensor.matmul(out=aggps[ci][:],
                                     lhsT=hsrc,
                                     rhs=at[:, ci * ICH:(ci + 1) * ICH],
                                     start=(j == 0), stop=(j == JT - 1),
                                     skip_group_check=True)
                # alternate the refills across both HWDGE rings (SP + ACT):
                # ACT is idle here, and two rings roughly double stream BW
                emit_adj(JT, eng=nc.scalar if (j & 1) else nc.sync)
            for ci in range(NI):
                nc.vector.tensor_copy(out=aggdT[:, ci * ICH:(ci + 1) * ICH],
                                      in_=aggps[ci][:])

        # ---------------- SAGE linear + pool + MLP ----------------
        with tc.tile_pool(name="mlpw", bufs=1) as mlpw, \
             tc.tile_pool(name="mlps", bufs=2) as mlps, \
             tc.tile_pool(name="mlpp", bufs=2, space="PSUM") as mlpp:
            WllT_t = mlpw.tile([F, F], f16, name="WllT_t")
            nc.sync.dma_start(out=WllT_t[:], in_=d['WllT'][:, :])
            WlrT_t = mlpw.tile([F, F], f16, name="WlrT_t")
            nc.sync.dma_start(out=WlrT_t[:], in_=d['WlrT'][:, :])
            bll_t = mlpw.tile([F, 1], f32, name="bll_t")
            nc.sync.dma_start(out=bll_t[:], in_=d['bll'][:, :])
            Wg1T_t = mlpw.tile([F, 1500], f32, name="Wg1T_t")
            nc.sync.dma_start(out=Wg1T_t[:], in_=d['Wg1T'][:, :])
            bg1_t = mlpw.tile([128, 12], f32, name="bg1_t")
            nc.sync.dma_start(out=bg1_t[:], in_=d['bg1'][:, :])
            Wg2_t = mlpw.tile([128, 12 * 128], f32, name="Wg2_t")
            nc.sync.dma_start(out=Wg2_t[:], in_=d['Wg2Tr'][:, :])
            bg2_t = mlpw.tile([128, 1], f32, name="bg2_t")
            nc.sync.dma_start(out=bg2_t[:], in_=d['bg2'][:, :])
            WoT_t = mlpw.tile([128, 1], f32, name="WoT_t")
            nc.sync.dma_start(out=WoT_t[:], in_=d['WoT'][:, :])

            for ci in range(NI):
                ps = mlpp.tile([F, ICH], f32, space="PSUM", tag="h2",
                               name="psh2")
                nc.tensor.matmul(out=ps[:], lhsT=WllT_t[:],
                                 rhs=aggdT[:, ci * ICH:(ci + 1) * ICH],
                                 start=True, stop=False, skip_group_check=True)
                nc.tensor.matmul(out=ps[:], lhsT=WlrT_t[:],
                                 rhs=hT[:, ci * ICH:(ci + 1) * ICH],
                                 start=False, stop=True, skip_group_check=True)
                nc.scalar.activation(out=h2T[:, ci * ICH:(ci + 1) * ICH],
                                     in_=ps[:], func=relu_f, bias=bll_t[:])

            gT = mlps.tile([F, GB], f32, name="gT")
            for g in range(GB):
                lo, hi = GRAPH_BOUNDS[g], GRAPH_BOUNDS[g + 1]
                nc.vector.tensor_reduce(out=gT[:, g:g + 1], in_=h2T[:, lo:hi],
                                        axis=mybir.AxisListType.X,
                                        op=mybir.AluOpType.max)
            g1T = mlps.tile([128, 12, GB], f32, name="g1T")
            for j in range(12):
                w = min(128, 1500 - j * 128)
                ps = mlpp.tile([128, GB], f32, space="PSUM", tag="g1",
                               name="psg1")
                nc.tensor.matmul(out=ps[:w, :],
                                 lhsT=Wg1T_t[:, j * 128:j * 128 + w],
                                 rhs=gT[:], start=True, stop=True)
                if w < 128:
                    nc.vector.memset(g1T[:, j, :], 0.0)
                nc.scalar.activation(out=g1T[:w, j, :], in_=ps[:w, :],
                                     func=relu_f, bias=bg1_t[:w, j:j + 1])
            g2ps = mlpp.tile([128, GB], f32, space="PSUM", tag="g2",
                             name="g2ps")
            for j in range(12):
                nc.tensor.matmul(out=g2ps[:],
                                 lhsT=Wg2_t[:, j * 128:(j + 1) * 128],
                                 rhs=g1T[:, j, :], start=(j == 0),
                                 stop=(j == 11), skip_group_check=True)
            g2sb = mlps.tile([128, GB], f32, name="g2sb")
            nc.vector.tensor_scalar_add(out=g2sb[:], in0=g2ps[:],
                                        scalar1=bg2_t[:])
            ops = mlpp.tile([1, GB], f32, space="PSUM", tag="o", name="ops")
            nc.tensor.matmul(out=ops[:], lhsT=WoT_t[:], rhs=g2sb[:],
                             start=True, stop=True)
            osb = mlps.tile([1, GB], f32, name="osb")
            nc.vector.tensor_scalar_add(out=osb[:], in0=ops[:],
                                        scalar1=float(d['bo_const']))
            nc.sync.dma_start(out=d['out8'][:, :], in_=osb[:])


def _build_program(bo_const):
    import concourse.tile as tile
    from concourse import bacc, mybir

    f32 = mybir.dt.float32
    f16 = mybir.dt.float16
    nc = bacc.Bacc("TRN2", target_bir_lowering=False, debug=False,
                   num_devices=NCORE)

    d = {}

    def dram_in(name, shape, dt=f32):
        d[name] = nc.dram_tensor(name, list(shape), dt, kind="ExternalInput")

    dram_in("QTd", (128, JPAD), f16)
    dram_in("KnTd", (128, IPAD), f16)
    dram_in("Vp", (128, JT, F1), f16)
    dram_in("Vl", (128, DBLK, F))
    dram_in("WllT", (F, F), f16)
    dram_in("WlrT", (F, F), f16)
    dram_in("bll", (F, 1))
    dram_in("Wg1T", (F, 1500))
    dram_in("bg1", (128, 12))
    dram_in("Wg2Tr", (128, 12 * 128))
    dram_in("bg2", (128, 1))
    dram_in("WoT", (128, 1))
    dram_in("ident", (128, 128))
    dram_in("Adj", (NPAD, IPAD), f16)
    d['out8'] = nc.dram_tensor("out8", [1, GB], f32, kind="ExternalOutput")
    d['h_loc'] = nc.dram_tensor("h_loc", [ROWS, HB], f16)
    d['h_full'] = nc.dram_tensor("h_full", [NPAD, HB], f16,
                                 addr_space="Shared")
    d['bo_const'] = bo_const

    with tile.TileContext(nc) as tc:
        _emit_body(nc, tc, d)

    nc.compile()
    return nc


# --------------------------------------------------------------------------
# entry point
# --------------------------------------------------------------------------

_CACHE = {}


def _make_in_maps(inputs):
    from concourse import mybir

    f16 = np.float16
    edge_index = np.asarray(inputs['edge_index'])
    w = _prep_weights(inputs)
    QTd, knts, Vp, vls = _host_qkv(inputs, f16)
    adjs = _prep_adj(edge_index, f16)
    ident = np.eye(128, dtype=np.float32)
    common = dict(
        QTd=QTd, Vp=Vp,
        WllT=w['WllT'].astype(f16), WlrT=w['WlrT'].astype(f16),
        bll=w['bll'], Wg1T=w['Wg1T'], bg1=w['bg1'], Wg2Tr=w['Wg2Tr'],
        bg2=w['bg2'], WoT=w['WoT'], ident=ident)
    in_maps = []
    for c in range(NCORE):
        m = dict(common)
        m['KnTd'] = knts[c]
        m['Vl'] = vls[c]
        m['Adj'] = adjs[c]
        in_maps.append(m)
    return in_maps, w['bo']


def kernel(**inputs):
    from concourse.bass_utils import run_bass_kernel_spmd

    in_maps, bo = _make_in_maps(inputs)
    key = ('prog', bo)
    if key not in _CACHE:
        _CACHE[key] = _build_program(bo)
    nc = _CACHE[key]

    res = run_bass_kernel_spmd(nc, in_maps, list(range(NCORE)))
    global LAST_RESULT
    LAST_RESULT = res
    out = np.zeros((B, 1), np.float32)
    for c in range(NCORE):
        out[c * GB:(c + 1) * GB, 0] = res.results[c]['out8'].reshape(-1)
    return out


LAST_RESULT = None
